# revision 1
# baseline (speedup 1.0000x reference)
"""LoRALinear (paged multi-adapter LoRA + base linear) Trainium2 kernel.

Full-input contract: kernel(**inputs) takes the unsharded tensors and
returns the full [T, D_OUT] output.

Sharding: tokens are split contiguously across the 8 NeuronCores
(1024 tokens/core).  The base weight, bias and the (tiny) LoRA page
caches are preprocessed on host into per-core dense operands:

  out_c = x_c @ W^T + bias + ((x_c @ A_c^T) * mask_c) @ B_c

where A_c/B_c stack the rank-64 page blocks of every adapter that
appears in core c's token range (G blocks, G==1 for the standard
equal-length-sequence layout) and mask_c[r, t] folds the per-token
adapter one-hot, the per-adapter rank mask and the per-sequence scaling
into one multiplier.  The bias is folded into the LoRA-B matmul as one
extra contraction row (ones row in the masked activations, bias row in
B).  All matmuls run as float32r (full PE rate for free dim >= 256).

Device schedule (per core):
  - x^T stays resident in SBUF (16 MiB); W^T streams through a small
    tile pool; per-k DMAs are interleaved (x slab, A slab, n=0 W tile)
    so the PE starts ~3us in instead of waiting for the full x load.
  - LoRA-A accumulators occupy PSUM banks until the x load finishes, so
    the n=0 output block only runs 6 of its 8 token-row tiles inline;
    the remaining 2 run as a deferred pass at the end (re-streaming
    n=0's W tiles, +8 MiB DMA, fully overlapped).
"""

import os

import numpy as np

import concourse.bass as bass
import concourse.bacc as bacc
import concourse.mybir as mybir
import concourse.tile as tile
from concourse.bass_utils import run_bass_kernel_spmd

N_CORES = 8
T = 8192
D_IN = 4096
D_OUT = 4096
TPC = T // N_CORES  # tokens per core
MAX_RANK = 64
P = 128
NFREE = 512  # matmul moving free dim (fp32 max)

F32 = mybir.dt.float32
F32R = mybir.dt.float32r

# exec time of the last device run (ns), when KERNEL_TRACE=1
last_exec_time_ns = None
last_results = None


def _rblocks(r_aug):
    """Split r_aug LoRA contraction rows into <=128-row blocks."""
    out = []
    start = 0
    while start < r_aug:
        cnt = min(P, r_aug - start)
        out.append((start, cnt))
        start += cnt
    return out


def _build_program(r_aug, d_in=D_IN, d_out=D_OUT, tpc=TPC,
                   w_bufs=12, o_bufs=4, out_dma="sync"):
    """Build the per-core Bass program.

    r_aug = G*64 + 1 LoRA contraction rows (last row = ones/bias).
    """
    k_tiles = d_in // P
    m_tiles = tpc // P
    n_tiles = d_out // NFREE
    t_chunks = tpc // NFREE
    rbs = _rblocks(r_aug)
    n_lora_ps = t_chunks * len(rbs)  # live LoRA-A psum tiles during startup
    # m-tiles of the n=0 block that fit alongside the LoRA-A accumulators
    m_inline = max(0, min(m_tiles, 8 - n_lora_ps))
    defer = list(range(m_inline, m_tiles))  # deferred to a tail pass

    nc = bacc.Bacc("TRN2", target_bir_lowering=False, debug=False)

    xT = nc.dram_tensor("xT", [d_in, tpc], F32R, kind="ExternalInput").ap()
    wT = nc.dram_tensor("wT", [d_in, d_out], F32R, kind="ExternalInput").ap()
    aT = nc.dram_tensor("aT", [d_in, r_aug], F32R, kind="ExternalInput").ap()
    bS = nc.dram_tensor("bS", [r_aug, d_out], F32R, kind="ExternalInput").ap()
    mS = nc.dram_tensor("mS", [r_aug, tpc], F32, kind="ExternalInput").ap()
    out = nc.dram_tensor("out", [tpc, d_out], F32, kind="ExternalOutput").ap()

    with tile.TileContext(nc) as tc:
        with (
            tc.tile_pool(name="xpool", bufs=k_tiles) as xpool,
            tc.tile_pool(name="cpool", bufs=1) as cpool,
            tc.tile_pool(name="wpool", bufs=w_bufs) as wpool,
            tc.tile_pool(name="opool", bufs=o_bufs) as opool,
            tc.tile_pool(name="psum", bufs=8, space="PSUM") as psum,
        ):
            # small resident inputs first (cheap DMAs, needed mid-flight)
            bss = {}
            mss = {}
            xam = {}
            for bi, (rs, rc) in enumerate(rbs):
                b_t = cpool.tile([rc, d_out], F32R, tag=f"bss{bi}",
                                 name=f"bss_{bi}")
                nc.sync.dma_start(b_t, bS[rs:rs + rc, :])
                bss[bi] = b_t
                m_t = cpool.tile([rc, tpc], F32, tag=f"mss{bi}",
                                 name=f"mss_{bi}")
                nc.sync.dma_start(m_t, mS[rs:rs + rc, :])
                mss[bi] = m_t
                xam[bi] = cpool.tile([rc, tpc], F32R, tag=f"xam{bi}",
                                     name=f"xam_{bi}")

            # ones/bias row lives in the last block's last row: copy it from
            # the mask now (also absorbs the mss DMA wait ahead of the muls).
            bl, (bl_rs, bl_rc) = len(rbs) - 1, rbs[-1]
            nc.vector.tensor_copy(xam[bl][bl_rc - 1:bl_rc, :],
                                  mss[bl][bl_rc - 1:bl_rc, :])

            # per-k interleaved loads: x slab, A slab, n=0 W tile.
            n0sl = slice(0, NFREE)
            xs = []
            ats = []
            wt0 = []
            for k in range(k_tiles):
                xt = xpool.tile([P, tpc], F32R, tag="xs", name=f"xs_{k}")
                nc.sync.dma_start(xt, xT[k * P:(k + 1) * P, :])
                xs.append(xt)
                at = cpool.tile([P, r_aug], F32R, tag="ats", bufs=k_tiles,
                                name=f"ats_{k}")
                nc.sync.dma_start(at, aT[k * P:(k + 1) * P, :])
                ats.append(at)
                wt = wpool.tile([P, NFREE], F32R, tag="wt", name=f"wt0_{k}")
                nc.sync.dma_start(wt, wT[k * P:(k + 1) * P, n0sl])
                wt0.append(wt)

            # LoRA-A accumulators: xamT[r, t] = sum_d A[r, d] x[t, d]
            lora_ps = {}
            for c in range(t_chunks):
                for bi, (rs, rc) in enumerate(rbs):
                    lora_ps[(c, bi)] = psum.tile([rc, NFREE], F32, tag="ps",
                                                 name=f"ps_lora_{c}_{bi}")
            # n=0 inline psum tiles
            psts0 = [psum.tile([P, NFREE], F32, tag="ps", name=f"pst_0_{i}")
                     for i in range(m_inline)]

            def copy_out(m, n, pst, idx):
                ot = opool.tile([P, NFREE], F32, tag="ot", name=f"ot_{n}_{m}")
                nc.vector.tensor_copy(ot, pst)
                # out_dma="scalar" rides the scalar engine's HWDGE queue so
                # stores don't sit behind the weight stream on the sync queue.
                getattr(nc, out_dma).dma_start(
                    out[m * P:(m + 1) * P, n * NFREE:(n + 1) * NFREE], ot)

            def lora_b(pst, m, nsl, stop):
                """Accumulate lora+bias rows into a base psum tile."""
                for bi, (rs, rc) in enumerate(rbs):
                    nc.tensor.matmul(
                        pst,
                        lhsT=xam[bi][:, m * P:(m + 1) * P],
                        rhs=bss[bi][:, nsl],
                        start=False,
                        stop=(stop and bi == len(rbs) - 1),
                    )

            # startup phase: per k, LoRA-A MMs + n=0 inline MMs
            for k in range(k_tiles):
                for c in range(t_chunks):
                    tsl = slice(c * NFREE, (c + 1) * NFREE)
                    for bi, (rs, rc) in enumerate(rbs):
                        nc.tensor.matmul(
                            lora_ps[(c, bi)],
                            lhsT=ats[k][:, rs:rs + rc],
                            rhs=xs[k][:, tsl],
                            start=(k == 0),
                            stop=(k == k_tiles - 1),
                        )
                for m in range(m_inline):
                    nc.tensor.matmul(
                        psts0[m],
                        lhsT=xs[k][:, m * P:(m + 1) * P],
                        rhs=wt0[k],
                        start=(k == 0),
                        stop=False,
                    )

            # masks: xam = lora_ps * mS (releases the LoRA psum tiles)
            for c in range(t_chunks):
                tsl = slice(c * NFREE, (c + 1) * NFREE)
                for bi, (rs, rc) in enumerate(rbs):
                    # last row of the last block is the ones row, keep it
                    rows = rc - 1 if bi == len(rbs) - 1 else rc
                    if rows:
                        nc.vector.tensor_mul(xam[bi][0:rows, tsl],
                                             lora_ps[(c, bi)][0:rows, :],
                                             mss[bi][0:rows, tsl])

            # finish n=0 inline m-tiles: lora rows + copy out
            for i, pst in enumerate(psts0):
                lora_b(pst, i, n0sl, stop=True)
                copy_out(i, 0, pst, i)

            # steady state: n = 1..n_tiles-1
            for n in range(1, n_tiles):
                nsl = slice(n * NFREE, (n + 1) * NFREE)
                psts = [psum.tile([P, NFREE], F32, tag="ps",
                                  name=f"pst_{n}_{i}") for i in range(m_tiles)]
                for k in range(k_tiles):
                    wt = wpool.tile([P, NFREE], F32R, tag="wt",
                                    name=f"wt_{n}_{k}")
                    nc.sync.dma_start(wt, wT[k * P:(k + 1) * P, nsl])
                    for m in range(m_tiles):
                        nc.tensor.matmul(
                            psts[m],
                            lhsT=xs[k][:, m * P:(m + 1) * P],
                            rhs=wt,
                            start=(k == 0),
                            stop=False,
                        )
                for m in range(m_tiles):
                    lora_b(psts[m], m, nsl, stop=True)
                    copy_out(m, n, psts[m], m)


            # deferred tail pass: n=0, m-tiles that were displaced by the
            # LoRA-A accumulators during startup (re-streams n=0 W tiles)
            if defer:
                pstd = [psum.tile([P, NFREE], F32, tag="ps",
                                  name=f"pstd_{i}") for i in defer]
                for k in range(k_tiles):
                    wt = wpool.tile([P, NFREE], F32R, tag="wt",
                                    name=f"wtd_{k}")
                    nc.sync.dma_start(wt, wT[k * P:(k + 1) * P, n0sl])
                    for j, m in enumerate(defer):
                        nc.tensor.matmul(
                            pstd[j],
                            lhsT=xs[k][:, m * P:(m + 1) * P],
                            rhs=wt,
                            start=(k == 0),
                            stop=False,
                        )
                for j, m in enumerate(defer):
                    lora_b(pstd[j], m, n0sl, stop=True)
                    copy_out(m, 0, pstd[j], j)

    nc.compile()
    return nc


def _prep_core_inputs(x, weight_t, bias, a_cache, b_cache, tok_adapter,
                      tok_scale, rank_page_table, ranks, core, g_max):
    """Host-side shard prep for one core."""
    d_in = x.shape[1]
    d_out = b_cache.shape[1]
    r = g_max * MAX_RANK
    sl = slice(core * TPC, (core + 1) * TPC)
    adapters = tok_adapter[sl]
    scales = tok_scale[sl]
    uniq = np.unique(adapters)

    aT = np.zeros((d_in, r + 1), np.float32)
    bS = np.zeros((r + 1, d_out), np.float32)
    mS = np.zeros((r + 1, TPC), np.float32)
    for g, a in enumerate(uniq):
        pages = rank_page_table[a]  # [64] page ids
        aT[:, g * MAX_RANK:(g + 1) * MAX_RANK] = a_cache[pages].T
        bS[g * MAX_RANK:(g + 1) * MAX_RANK, :] = b_cache[pages]
        slot_active = (np.arange(MAX_RANK) < ranks[a])[:, None]  # [64, 1]
        tok_active = (adapters == a)[None, :]  # [1, TPC]
        mS[g * MAX_RANK:(g + 1) * MAX_RANK, :] = (
            slot_active & tok_active) * scales[None, :]
    bS[r, :] = bias
    mS[r, :] = 1.0
    xT = np.ascontiguousarray(x[sl].T)
    return {"xT": xT, "wT": weight_t, "aT": np.ascontiguousarray(aT),
            "bS": bS, "mS": mS}


def kernel(x, weight, bias, a_cache, b_cache, b_start_loc, b_adapter_ids,
           b_scaling, rank_page_table, ranks):
    global last_exec_time_ns, last_results
    x = np.asarray(x, np.float32)
    weight = np.asarray(weight, np.float32)
    bias = np.asarray(bias, np.float32)
    a_cache = np.asarray(a_cache, np.float32)
    b_cache = np.asarray(b_cache, np.float32)
    b_start_loc = np.asarray(b_start_loc)
    b_adapter_ids = np.asarray(b_adapter_ids)
    b_scaling = np.asarray(b_scaling, np.float32)
    rank_page_table = np.asarray(rank_page_table)
    ranks = np.asarray(ranks)

    t = x.shape[0]
    seg = np.searchsorted(b_start_loc, np.arange(t, dtype=b_start_loc.dtype),
                          side="right") - 1
    tok_adapter = b_adapter_ids[seg]
    tok_scale = b_scaling[seg]

    g_max = max(
        len(np.unique(tok_adapter[c * TPC:(c + 1) * TPC]))
        for c in range(N_CORES)
    )
    r_aug = g_max * MAX_RANK + 1

    weight_t = np.ascontiguousarray(weight.T)
    in_maps = [
        _prep_core_inputs(x, weight_t, bias, a_cache, b_cache, tok_adapter,
                          tok_scale, rank_page_table, ranks, c, g_max)
        for c in range(N_CORES)
    ]

    nc = _build_program(r_aug)
    trace = os.environ.get("KERNEL_TRACE", "0") == "1"
    repeat = int(os.environ.get("KERNEL_REPEAT", "1"))
    times = []
    for _ in range(repeat):
        res = run_bass_kernel_spmd(nc, in_maps, core_ids=list(range(N_CORES)),
                                   trace=trace)
        times.append(res.exec_time_ns)
    last_exec_time_ns = (min(t for t in times if t is not None)
                         if any(t is not None for t in times) else None)
    last_results = res
    if repeat > 1:
        print("exec times:", times)
    return np.concatenate([res.results[c]["out"] for c in range(N_CORES)],
                          axis=0).astype(np.float32)



# revision 2
# speedup vs baseline: 1.1311x; 1.1311x over previous
"""LoRALinear (paged multi-adapter LoRA + base linear) Trainium2 kernel.

Full-input contract: kernel(**inputs) takes the unsharded tensors and
returns the full [T, D_OUT] output.

Sharding: tokens are split contiguously across the 8 NeuronCores
(1024 tokens/core).  The base weight, bias and the (tiny) LoRA page
caches are preprocessed on host into per-core dense operands:

  out_c = x_c @ W^T + bias + ((x_c @ A_c^T) * mask_c) @ B_c

where A_c/B_c stack the rank-64 page blocks of every adapter that
appears in core c's token range (G blocks, G==1 for the standard
equal-length-sequence layout) and mask_c[r, t] folds the per-token
adapter one-hot, the per-adapter rank mask and the per-sequence scaling
into one multiplier.  The bias is folded into the LoRA-B matmul as one
extra contraction row (ones row in the masked activations, bias row in
B).

Matmul inputs are bf16 (PSUM accumulation stays fp32): the PE runs at
the same 1 cycle/row as fp32r, but every weight/activation DMA halves,
which removes the HBM-bound startup stalls the fp32r version had.

Device schedule (per core):
  - x^T stays resident in SBUF (8 MiB bf16); W^T streams through a
    32-tile pool (one full n-block of prefetch); per-k DMAs are
    interleaved (x slab, A slab, n=0 W tile) so the PE starts early.
  - LoRA-A accumulators occupy 2 PSUM banks until the x load finishes,
    so the n=0 output block only runs 6 of its 8 token-row tiles
    inline; the remaining 2 run right after from the n=0 W tiles which
    are kept resident in SBUF (no re-stream).
  - Output stores ride the scalar engine's HWDGE queue so they never
    head-of-line block the weight stream on the sync queue.
"""

import os

import numpy as np
import ml_dtypes

import concourse.bass as bass
import concourse.bacc as bacc
import concourse.mybir as mybir
import concourse.tile as tile
from concourse.bass_utils import run_bass_kernel_spmd

N_CORES = 8
T = 8192
D_IN = 4096
D_OUT = 4096
TPC = T // N_CORES  # tokens per core
MAX_RANK = 64
P = 128
NFREE = 512  # matmul moving free dim (PSUM bank)

F32 = mybir.dt.float32
BF16 = mybir.dt.bfloat16
NP_BF16 = ml_dtypes.bfloat16

# exec time of the last device run (ns), when KERNEL_TRACE=1
last_exec_time_ns = None
last_results = None


def _rblocks(r_aug):
    """Split r_aug LoRA contraction rows into <=128-row blocks."""
    out = []
    start = 0
    while start < r_aug:
        cnt = min(P, r_aug - start)
        out.append((start, cnt))
        start += cnt
    return out


def _build_program(r_aug, d_in=D_IN, d_out=D_OUT, tpc=TPC,
                   w_bufs=32, o_bufs=8):
    """Build the per-core Bass program.

    r_aug = G*64 + 1 LoRA contraction rows (last row = ones/bias).
    """
    k_tiles = d_in // P
    m_tiles = tpc // P
    n_tiles = d_out // NFREE
    t_chunks = tpc // NFREE
    rbs = _rblocks(r_aug)
    n_lora_ps = t_chunks * len(rbs)  # live LoRA-A psum tiles during startup
    # m-tiles of the n=0 block that fit alongside the LoRA-A accumulators
    m_inline = max(0, min(m_tiles, 8 - n_lora_ps))
    defer = list(range(m_inline, m_tiles))  # run right after, from SBUF

    nc = bacc.Bacc("TRN2", target_bir_lowering=False, debug=False)

    xT = nc.dram_tensor("xT", [d_in, tpc], BF16, kind="ExternalInput").ap()
    wT = nc.dram_tensor("wT", [d_in, d_out], BF16, kind="ExternalInput").ap()
    aT = nc.dram_tensor("aT", [d_in, r_aug], BF16, kind="ExternalInput").ap()
    bS = nc.dram_tensor("bS", [r_aug, d_out], BF16, kind="ExternalInput").ap()
    mS = nc.dram_tensor("mS", [r_aug, tpc], F32, kind="ExternalInput").ap()
    out = nc.dram_tensor("out", [tpc, d_out], F32, kind="ExternalOutput").ap()

    with tile.TileContext(nc) as tc:
        with (
            tc.tile_pool(name="xpool", bufs=k_tiles) as xpool,
            tc.tile_pool(name="cpool", bufs=1) as cpool,
            tc.tile_pool(name="wpool", bufs=w_bufs) as wpool,
            tc.tile_pool(name="opool", bufs=o_bufs) as opool,
            tc.tile_pool(name="psum", bufs=8, space="PSUM") as psum,
        ):
            n0sl = slice(0, NFREE)
            xs = []
            ats = []
            wt0 = []

            def load_k(k):
                """Interleaved per-k loads: x slab, A slab, n=0 W tile.

                The n=0 W tiles are kept resident for the deferred
                m-tiles, so n=0 is never re-streamed.
                """
                xt = xpool.tile([P, tpc], BF16, tag="xs", name=f"xs_{k}")
                nc.sync.dma_start(xt, xT[k * P:(k + 1) * P, :])
                xs.append(xt)
                at = cpool.tile([P, r_aug], BF16, tag="ats", bufs=k_tiles,
                                name=f"ats_{k}")
                nc.sync.dma_start(at, aT[k * P:(k + 1) * P, :])
                ats.append(at)
                wt = cpool.tile([P, NFREE], BF16, tag="wt0", bufs=k_tiles,
                                name=f"wt0_{k}")
                nc.sync.dma_start(wt, wT[k * P:(k + 1) * P, n0sl])
                wt0.append(wt)

            # first compute tile's operands go in front of everything
            load_k(0)

            # small resident inputs next (cheap DMAs, needed mid-flight)
            bss = {}
            mss = {}
            xam = {}
            for bi, (rs, rc) in enumerate(rbs):
                b_t = cpool.tile([rc, d_out], BF16, tag=f"bss{bi}",
                                 name=f"bss_{bi}")
                nc.sync.dma_start(b_t, bS[rs:rs + rc, :])
                bss[bi] = b_t
                m_t = cpool.tile([rc, tpc], F32, tag=f"mss{bi}",
                                 name=f"mss_{bi}")
                nc.sync.dma_start(m_t, mS[rs:rs + rc, :])
                mss[bi] = m_t
                xam[bi] = cpool.tile([rc, tpc], BF16, tag=f"xam{bi}",
                                     name=f"xam_{bi}")

            # ones/bias row lives in the last block's last row: copy it from
            # the mask now (also absorbs the mss DMA wait ahead of the muls).
            bl, (bl_rs, bl_rc) = len(rbs) - 1, rbs[-1]
            nc.vector.tensor_copy(xam[bl][bl_rc - 1:bl_rc, :],
                                  mss[bl][bl_rc - 1:bl_rc, :])

            for k in range(1, k_tiles):
                load_k(k)

            # LoRA-A accumulators: xamT[r, t] = sum_d A[r, d] x[t, d]
            lora_ps = {}
            for c in range(t_chunks):
                for bi, (rs, rc) in enumerate(rbs):
                    lora_ps[(c, bi)] = psum.tile([rc, NFREE], F32, tag="ps",
                                                 name=f"ps_lora_{c}_{bi}")
            # n=0 inline psum tiles
            psts0 = [psum.tile([P, NFREE], F32, tag="ps", name=f"pst_0_{i}")
                     for i in range(m_inline)]

            def copy_out(m, n, pst, idx):
                ot = opool.tile([P, NFREE], F32, tag="ot", name=f"ot_{n}_{m}")
                nc.vector.tensor_copy(ot, pst)
                # stores ride the scalar engine's HWDGE queue so they don't
                # sit in front of the weight stream on the sync queue.
                nc.scalar.dma_start(
                    out[m * P:(m + 1) * P, n * NFREE:(n + 1) * NFREE], ot)

            def lora_b(pst, m, nsl, stop):
                """Accumulate lora+bias rows into a base psum tile."""
                for bi, (rs, rc) in enumerate(rbs):
                    nc.tensor.matmul(
                        pst,
                        lhsT=xam[bi][:, m * P:(m + 1) * P],
                        rhs=bss[bi][:, nsl],
                        start=False,
                        stop=(stop and bi == len(rbs) - 1),
                    )

            # startup phase: per k, LoRA-A MMs + n=0 inline MMs
            for k in range(k_tiles):
                for c in range(t_chunks):
                    tsl = slice(c * NFREE, (c + 1) * NFREE)
                    for bi, (rs, rc) in enumerate(rbs):
                        nc.tensor.matmul(
                            lora_ps[(c, bi)],
                            lhsT=ats[k][:, rs:rs + rc],
                            rhs=xs[k][:, tsl],
                            start=(k == 0),
                            stop=(k == k_tiles - 1),
                        )
                for m in range(m_inline):
                    nc.tensor.matmul(
                        psts0[m],
                        lhsT=xs[k][:, m * P:(m + 1) * P],
                        rhs=wt0[k],
                        start=(k == 0),
                        stop=False,
                    )

            # masks: xam = lora_ps * mS (releases the LoRA psum tiles)
            for c in range(t_chunks):
                tsl = slice(c * NFREE, (c + 1) * NFREE)
                for bi, (rs, rc) in enumerate(rbs):
                    # last row of the last block is the ones row, keep it
                    rows = rc - 1 if bi == len(rbs) - 1 else rc
                    if rows:
                        nc.vector.tensor_mul(xam[bi][0:rows, tsl],
                                             lora_ps[(c, bi)][0:rows, :],
                                             mss[bi][0:rows, tsl])

            # finish n=0 inline m-tiles: lora rows + copy out
            for i, pst in enumerate(psts0):
                lora_b(pst, i, n0sl, stop=True)
                copy_out(i, 0, pst, i)

            # deferred n=0 m-tiles (displaced by the LoRA-A accumulators
            # during startup): everything is resident in SBUF, no DMA.
            if defer:
                pstd = [psum.tile([P, NFREE], F32, tag="ps",
                                  name=f"pstd_{i}") for i in defer]
                for k in range(k_tiles):
                    for j, m in enumerate(defer):
                        nc.tensor.matmul(
                            pstd[j],
                            lhsT=xs[k][:, m * P:(m + 1) * P],
                            rhs=wt0[k],
                            start=(k == 0),
                            stop=False,
                        )
                for j, m in enumerate(defer):
                    lora_b(pstd[j], m, n0sl, stop=True)
                    copy_out(m, 0, pstd[j], j)

            # steady state: n = 1..n_tiles-1
            for n in range(1, n_tiles):
                nsl = slice(n * NFREE, (n + 1) * NFREE)
                psts = [psum.tile([P, NFREE], F32, tag="ps",
                                  name=f"pst_{n}_{i}") for i in range(m_tiles)]
                for k in range(k_tiles):
                    wt = wpool.tile([P, NFREE], BF16, tag="wt",
                                    name=f"wt_{n}_{k}")
                    nc.sync.dma_start(wt, wT[k * P:(k + 1) * P, nsl])
                    for m in range(m_tiles):
                        nc.tensor.matmul(
                            psts[m],
                            lhsT=xs[k][:, m * P:(m + 1) * P],
                            rhs=wt,
                            start=(k == 0),
                            stop=False,
                        )
                for m in range(m_tiles):
                    lora_b(psts[m], m, nsl, stop=True)
                    copy_out(m, n, psts[m], m)

    nc.compile()
    return nc


def _prep_core_inputs(x, weight_t, bias, a_cache, b_cache, tok_adapter,
                      tok_scale, rank_page_table, ranks, core, g_max):
    """Host-side shard prep for one core."""
    d_in = x.shape[1]
    d_out = b_cache.shape[1]
    r = g_max * MAX_RANK
    sl = slice(core * TPC, (core + 1) * TPC)
    adapters = tok_adapter[sl]
    scales = tok_scale[sl]
    uniq = np.unique(adapters)

    aT = np.zeros((d_in, r + 1), NP_BF16)
    bS = np.zeros((r + 1, d_out), NP_BF16)
    mS = np.zeros((r + 1, TPC), np.float32)
    for g, a in enumerate(uniq):
        pages = rank_page_table[a]  # [64] page ids
        aT[:, g * MAX_RANK:(g + 1) * MAX_RANK] = a_cache[pages].T
        bS[g * MAX_RANK:(g + 1) * MAX_RANK, :] = b_cache[pages]
        slot_active = (np.arange(MAX_RANK) < ranks[a])[:, None]  # [64, 1]
        tok_active = (adapters == a)[None, :]  # [1, TPC]
        mS[g * MAX_RANK:(g + 1) * MAX_RANK, :] = (
            slot_active & tok_active) * scales[None, :]
    bS[r, :] = bias
    mS[r, :] = 1.0
    xT = np.ascontiguousarray(x[sl].T).astype(NP_BF16)
    return {"xT": xT, "wT": weight_t, "aT": np.ascontiguousarray(aT),
            "bS": bS, "mS": mS}


def kernel(x, weight, bias, a_cache, b_cache, b_start_loc, b_adapter_ids,
           b_scaling, rank_page_table, ranks):
    global last_exec_time_ns, last_results
    x = np.asarray(x, np.float32)
    weight = np.asarray(weight, np.float32)
    bias = np.asarray(bias, np.float32)
    a_cache = np.asarray(a_cache, np.float32)
    b_cache = np.asarray(b_cache, np.float32)
    b_start_loc = np.asarray(b_start_loc)
    b_adapter_ids = np.asarray(b_adapter_ids)
    b_scaling = np.asarray(b_scaling, np.float32)
    rank_page_table = np.asarray(rank_page_table)
    ranks = np.asarray(ranks)

    t = x.shape[0]
    seg = np.searchsorted(b_start_loc, np.arange(t, dtype=b_start_loc.dtype),
                          side="right") - 1
    tok_adapter = b_adapter_ids[seg]
    tok_scale = b_scaling[seg]

    g_max = max(
        len(np.unique(tok_adapter[c * TPC:(c + 1) * TPC]))
        for c in range(N_CORES)
    )
    r_aug = g_max * MAX_RANK + 1

    weight_t = np.ascontiguousarray(weight.T).astype(NP_BF16)
    in_maps = [
        _prep_core_inputs(x, weight_t, bias, a_cache, b_cache, tok_adapter,
                          tok_scale, rank_page_table, ranks, c, g_max)
        for c in range(N_CORES)
    ]

    nc = _build_program(r_aug)
    trace = os.environ.get("KERNEL_TRACE", "0") == "1"
    repeat = int(os.environ.get("KERNEL_REPEAT", "1"))
    times = []
    for _ in range(repeat):
        res = run_bass_kernel_spmd(nc, in_maps, core_ids=list(range(N_CORES)),
                                   trace=trace)
        times.append(res.exec_time_ns)
    last_exec_time_ns = (min(t for t in times if t is not None)
                         if any(t is not None for t in times) else None)
    last_results = res
    if repeat > 1:
        print("exec times:", times)
    return np.concatenate([res.results[c]["out"] for c in range(N_CORES)],
                          axis=0).astype(np.float32)


# revision 6
# speedup vs baseline: 1.1380x; 1.0062x over previous
"""LoRALinear (paged multi-adapter LoRA + base linear) Trainium2 kernel.

Full-input contract: kernel(**inputs) takes the unsharded tensors and
returns the full [T, D_OUT] output.

Sharding: tokens are split contiguously across the 8 NeuronCores
(1024 tokens/core).  The base weight, bias and the (tiny) LoRA page
caches are preprocessed on host into per-core dense operands:

  out_c = x_c @ W^T + bias + ((x_c @ A_c^T) * mask_c) @ B_c

where A_c/B_c stack the rank-64 page blocks of every adapter that
appears in core c's token range (G blocks, G==1 for the standard
equal-length-sequence layout) and mask_c[r, t] folds the per-token
adapter one-hot, the per-adapter rank mask and the per-sequence scaling
into one multiplier.  The bias is folded into the LoRA-B matmul as one
extra contraction row (ones row in the masked activations, bias row in
B).

Matmul inputs are bf16 (PSUM accumulation stays fp32): the PE runs at
the same 1 cycle/row as fp32r, but every weight/activation DMA halves,
which removes the HBM-bound startup stalls the fp32r version had.

Device schedule (per core):
  - x^T stays resident in SBUF (8 MiB bf16); W^T streams through a
    32-tile pool (one full n-block of prefetch); per-k DMAs are
    interleaved (x slab, A slab, n=0 W tile) so the PE starts early.
  - LoRA-A accumulators occupy 2 PSUM banks until the x load finishes,
    so the n=0 output block only runs 6 of its 8 token-row tiles
    inline; the remaining 2 run right after from the n=0 W tiles which
    are kept resident in SBUF (no re-stream).
  - Output stores ride the scalar engine's HWDGE queue so they never
    head-of-line block the weight stream on the sync queue.
"""

import os

import numpy as np
import ml_dtypes

import concourse.bass as bass
import concourse.bacc as bacc
import concourse.mybir as mybir
import concourse.tile as tile
from concourse.bass_utils import run_bass_kernel_spmd

N_CORES = 8
T = 8192
D_IN = 4096
D_OUT = 4096
TPC = T // N_CORES  # tokens per core
MAX_RANK = 64
P = 128
NFREE = 512  # matmul moving free dim (PSUM bank)

F32 = mybir.dt.float32
BF16 = mybir.dt.bfloat16
NP_BF16 = ml_dtypes.bfloat16

# exec time of the last device run (ns), when KERNEL_TRACE=1
last_exec_time_ns = None
last_results = None


def _rblocks(r_aug):
    """Split r_aug LoRA contraction rows into <=128-row blocks."""
    out = []
    start = 0
    while start < r_aug:
        cnt = min(P, r_aug - start)
        out.append((start, cnt))
        start += cnt
    return out


def _build_program(r_aug, d_in=D_IN, d_out=D_OUT, tpc=TPC,
                   w_bufs=64, o_bufs=4):
    """Build the per-core Bass program.

    r_aug = G*64 + 1 LoRA contraction rows (last row = ones/bias).
    """
    k_tiles = d_in // P
    m_tiles = tpc // P
    n_tiles = d_out // NFREE
    t_chunks = tpc // NFREE
    rbs = _rblocks(r_aug)
    n_lora_ps = t_chunks * len(rbs)  # live LoRA-A psum tiles during startup
    # m-tiles of the n=0 block that fit alongside the LoRA-A accumulators
    m_inline = max(0, min(m_tiles, 8 - n_lora_ps))
    defer = list(range(m_inline, m_tiles))  # run right after, from SBUF

    nc = bacc.Bacc("TRN2", target_bir_lowering=False, debug=False)

    xT = nc.dram_tensor("xT", [d_in, tpc], BF16, kind="ExternalInput").ap()
    wT = nc.dram_tensor("wT", [d_in, d_out], BF16, kind="ExternalInput").ap()
    aT = nc.dram_tensor("aT", [d_in, r_aug], BF16, kind="ExternalInput").ap()
    bS = nc.dram_tensor("bS", [r_aug, d_out], BF16, kind="ExternalInput").ap()
    mS = nc.dram_tensor("mS", [r_aug, tpc], F32, kind="ExternalInput").ap()
    out = nc.dram_tensor("out", [tpc, d_out], F32, kind="ExternalOutput").ap()

    with tile.TileContext(nc) as tc:
        with (
            tc.tile_pool(name="xpool", bufs=k_tiles) as xpool,
            tc.tile_pool(name="cpool", bufs=1) as cpool,
            tc.tile_pool(name="wpool", bufs=w_bufs) as wpool,
            tc.tile_pool(name="opool", bufs=o_bufs) as opool,
            tc.tile_pool(name="psum", bufs=8, space="PSUM") as psum,
        ):
            n0sl = slice(0, NFREE)
            xs = []
            ats = []
            wt0 = []

            def load_k(k):
                """Interleaved per-k loads: x slab, A slab, n=0 W tile.

                The n=0 W tiles are kept resident for the deferred
                m-tiles, so n=0 is never re-streamed.
                """
                xt = xpool.tile([P, tpc], BF16, tag="xs", name=f"xs_{k}")
                nc.sync.dma_start(xt, xT[k * P:(k + 1) * P, :])
                xs.append(xt)
                at = cpool.tile([P, r_aug], BF16, tag="ats", bufs=k_tiles,
                                name=f"ats_{k}")
                nc.sync.dma_start(at, aT[k * P:(k + 1) * P, :])
                ats.append(at)
                wt = cpool.tile([P, NFREE], BF16, tag="wt0", bufs=k_tiles,
                                name=f"wt0_{k}")
                nc.sync.dma_start(wt, wT[k * P:(k + 1) * P, n0sl])
                wt0.append(wt)

            # first compute tile's operands go in front of everything
            load_k(0)

            # small resident inputs on the gpsimd engine's DMA queue so they
            # never delay the x/A/W stream on the sync queue.
            bss = {}
            mss = {}
            xam = {}
            for bi, (rs, rc) in enumerate(rbs):
                b_t = cpool.tile([rc, d_out], BF16, tag=f"bss{bi}",
                                 name=f"bss_{bi}")
                nc.gpsimd.dma_start(b_t, bS[rs:rs + rc, :])
                bss[bi] = b_t
                m_t = cpool.tile([rc, tpc], F32, tag=f"mss{bi}",
                                 name=f"mss_{bi}")
                nc.gpsimd.dma_start(m_t, mS[rs:rs + rc, :])
                mss[bi] = m_t
                xam[bi] = cpool.tile([rc, tpc], BF16, tag=f"xam{bi}",
                                     name=f"xam_{bi}")

            # ones/bias row lives in the last block's last row: copy it from
            # the mask now (also absorbs the mss DMA wait ahead of the muls).
            bl, (bl_rs, bl_rc) = len(rbs) - 1, rbs[-1]
            nc.vector.tensor_copy(xam[bl][bl_rc - 1:bl_rc, :],
                                  mss[bl][bl_rc - 1:bl_rc, :])

            for k in range(1, k_tiles):
                load_k(k)

            # LoRA-A accumulators: xamT[r, t] = sum_d A[r, d] x[t, d]
            lora_ps = {}
            for c in range(t_chunks):
                for bi, (rs, rc) in enumerate(rbs):
                    lora_ps[(c, bi)] = psum.tile([rc, NFREE], F32, tag="ps",
                                                 name=f"ps_lora_{c}_{bi}")
            # n=0 inline psum tiles
            psts0 = [psum.tile([P, NFREE], F32, tag="ps", name=f"pst_0_{i}")
                     for i in range(m_inline)]

            def copy_out(m, n, pst, idx):
                ot = opool.tile([P, NFREE], F32, tag="ot", name=f"ot_{n}_{m}")
                nc.vector.tensor_copy(ot, pst)
                # stores ride the scalar engine's HWDGE queue so they don't
                # sit in front of the weight stream on the sync queue.
                nc.scalar.dma_start(
                    out[m * P:(m + 1) * P, n * NFREE:(n + 1) * NFREE], ot)

            def lora_b(pst, m, nsl, stop):
                """Accumulate lora+bias rows into a base psum tile."""
                for bi, (rs, rc) in enumerate(rbs):
                    nc.tensor.matmul(
                        pst,
                        lhsT=xam[bi][:, m * P:(m + 1) * P],
                        rhs=bss[bi][:, nsl],
                        start=False,
                        stop=(stop and bi == len(rbs) - 1),
                    )

            # startup phase: per k, LoRA-A MMs + n=0 inline MMs
            for k in range(k_tiles):
                for c in range(t_chunks):
                    tsl = slice(c * NFREE, (c + 1) * NFREE)
                    for bi, (rs, rc) in enumerate(rbs):
                        nc.tensor.matmul(
                            lora_ps[(c, bi)],
                            lhsT=ats[k][:, rs:rs + rc],
                            rhs=xs[k][:, tsl],
                            start=(k == 0),
                            stop=(k == k_tiles - 1),
                        )
                for m in range(m_inline):
                    nc.tensor.matmul(
                        psts0[m],
                        lhsT=xs[k][:, m * P:(m + 1) * P],
                        rhs=wt0[k],
                        start=(k == 0),
                        stop=False,
                    )

            # masks: xam = lora_ps * mS (releases the LoRA psum tiles)
            for c in range(t_chunks):
                tsl = slice(c * NFREE, (c + 1) * NFREE)
                for bi, (rs, rc) in enumerate(rbs):
                    # last row of the last block is the ones row, keep it
                    rows = rc - 1 if bi == len(rbs) - 1 else rc
                    if rows:
                        nc.vector.tensor_mul(xam[bi][0:rows, tsl],
                                             lora_ps[(c, bi)][0:rows, :],
                                             mss[bi][0:rows, tsl])

            # finish n=0 inline m-tiles: lora rows + copy out
            for i, pst in enumerate(psts0):
                lora_b(pst, i, n0sl, stop=True)
                copy_out(i, 0, pst, i)

            # deferred n=0 m-tiles (displaced by the LoRA-A accumulators
            # during startup): everything is resident in SBUF, no DMA.
            for m in defer:
                pst = psum.tile([P, NFREE], F32, tag="ps", name=f"pstd_{m}")
                for k in range(k_tiles):
                    nc.tensor.matmul(
                        pst,
                        lhsT=xs[k][:, m * P:(m + 1) * P],
                        rhs=wt0[k],
                        start=(k == 0),
                        stop=False,
                    )
                lora_b(pst, m, n0sl, stop=True)
                copy_out(m, 0, pst, m)

            # steady state: n = 1..n_tiles-1, m-outer / k-inner.  Each W
            # block is fully prefetched one block ahead (w_bufs covers two
            # blocks), so each m-tile finishes its accumulation 1/8th of a
            # block apart and the psum copies + output stores spread evenly
            # instead of bunching at block boundaries (which made an ~8us
            # store-drain tail after the last matmul).
            for n in range(1, n_tiles):
                nsl = slice(n * NFREE, (n + 1) * NFREE)
                wts = []
                for k in range(k_tiles):
                    wt = wpool.tile([P, NFREE], BF16, tag="wt",
                                    name=f"wt_{n}_{k}")
                    nc.sync.dma_start(wt, wT[k * P:(k + 1) * P, nsl])
                    wts.append(wt)
                for m in range(m_tiles):
                    pst = psum.tile([P, NFREE], F32, tag="ps",
                                    name=f"pst_{n}_{m}")
                    for k in range(k_tiles):
                        nc.tensor.matmul(
                            pst,
                            lhsT=xs[k][:, m * P:(m + 1) * P],
                            rhs=wts[k],
                            start=(k == 0),
                            stop=False,
                        )
                    lora_b(pst, m, nsl, stop=True)
                    copy_out(m, n, pst, m)

    nc.compile()
    return nc


def _prep_core_inputs(x, weight_t, bias, a_cache, b_cache, tok_adapter,
                      tok_scale, rank_page_table, ranks, core, g_max):
    """Host-side shard prep for one core."""
    d_in = x.shape[1]
    d_out = b_cache.shape[1]
    r = g_max * MAX_RANK
    sl = slice(core * TPC, (core + 1) * TPC)
    adapters = tok_adapter[sl]
    scales = tok_scale[sl]
    uniq = np.unique(adapters)

    aT = np.zeros((d_in, r + 1), NP_BF16)
    bS = np.zeros((r + 1, d_out), NP_BF16)
    mS = np.zeros((r + 1, TPC), np.float32)
    for g, a in enumerate(uniq):
        pages = rank_page_table[a]  # [64] page ids
        aT[:, g * MAX_RANK:(g + 1) * MAX_RANK] = a_cache[pages].T
        bS[g * MAX_RANK:(g + 1) * MAX_RANK, :] = b_cache[pages]
        slot_active = (np.arange(MAX_RANK) < ranks[a])[:, None]  # [64, 1]
        tok_active = (adapters == a)[None, :]  # [1, TPC]
        mS[g * MAX_RANK:(g + 1) * MAX_RANK, :] = (
            slot_active & tok_active) * scales[None, :]
    bS[r, :] = bias
    mS[r, :] = 1.0
    xT = np.ascontiguousarray(x[sl].T).astype(NP_BF16)
    return {"xT": xT, "wT": weight_t, "aT": np.ascontiguousarray(aT),
            "bS": bS, "mS": mS}


def kernel(x, weight, bias, a_cache, b_cache, b_start_loc, b_adapter_ids,
           b_scaling, rank_page_table, ranks):
    global last_exec_time_ns, last_results
    x = np.asarray(x, np.float32)
    weight = np.asarray(weight, np.float32)
    bias = np.asarray(bias, np.float32)
    a_cache = np.asarray(a_cache, np.float32)
    b_cache = np.asarray(b_cache, np.float32)
    b_start_loc = np.asarray(b_start_loc)
    b_adapter_ids = np.asarray(b_adapter_ids)
    b_scaling = np.asarray(b_scaling, np.float32)
    rank_page_table = np.asarray(rank_page_table)
    ranks = np.asarray(ranks)

    t = x.shape[0]
    seg = np.searchsorted(b_start_loc, np.arange(t, dtype=b_start_loc.dtype),
                          side="right") - 1
    tok_adapter = b_adapter_ids[seg]
    tok_scale = b_scaling[seg]

    g_max = max(
        len(np.unique(tok_adapter[c * TPC:(c + 1) * TPC]))
        for c in range(N_CORES)
    )
    r_aug = g_max * MAX_RANK + 1

    weight_t = np.ascontiguousarray(weight.T).astype(NP_BF16)
    in_maps = [
        _prep_core_inputs(x, weight_t, bias, a_cache, b_cache, tok_adapter,
                          tok_scale, rank_page_table, ranks, c, g_max)
        for c in range(N_CORES)
    ]

    nc = _build_program(r_aug)
    trace = os.environ.get("KERNEL_TRACE", "0") == "1"
    repeat = int(os.environ.get("KERNEL_REPEAT", "1"))
    times = []
    for _ in range(repeat):
        res = run_bass_kernel_spmd(nc, in_maps, core_ids=list(range(N_CORES)),
                                   trace=trace)
        times.append(res.exec_time_ns)
    last_exec_time_ns = (min(t for t in times if t is not None)
                         if any(t is not None for t in times) else None)
    last_results = res
    if repeat > 1:
        print("exec times:", times)
    return np.concatenate([res.results[c]["out"] for c in range(N_CORES)],
                          axis=0).astype(np.float32)
